# revision 36
# baseline (speedup 1.0000x reference)
"""Trainium2 Bass kernel for nn_ClassificationLayer (Gaussian pdf-sum classifier).

Math:
  mu/sd per dim from tiny [128,10] reference sets (host, exact).
  Per row i: s_n[i] = sum_d INV_SQRT_2PI/sd_d * exp(-0.5*((x[i,d]-mu_d)/sd_d)^2)
  (same for anomaly), then the batch recurrence p_k = (p_{k-1} + s_k)/128,
  output = [pn/(pn+pa), pa/(pn+pa)].

Device strategy (8 cores, data-parallel over N):
  - Host transposes each core's row-shard to [128 dims, R rows] fp16 so
    per-dim constants become per-partition scale/bias, and HBM traffic
    halves vs fp32 (DMA floor ~45us/core).
  - Row-columns are split between two engines working concurrently:
      * ScalarE ACTIVATE Derivative_Erf(scale*x+bias) = (2/sqrt(pi))exp(-z^2)
        -> exact Gaussian, fp16 out (~0.88 ns/col/dist).
      * VectorE custom DVE op GAUSS_BITS_ANT: relu(C2 - (A*x+B)^2) -> int16,
        which IS the fp16 bit pattern of ~(2/sqrt(pi))exp(-z^2) (Schraudolph
        exp2 trick, ~3% element ripple that averages out over 128 dims;
        end-to-end rel err ~1e-3). One DVE instr per dist (~1.07 ns/col/dist).
  - Reduction over dims (partitions) via fp16 TensorE matvec; the stationary
    operand is a 64-wide shifted window over a zero-padded fp16 weight buffer
    so chunk g's sums land in PSUM partition g%64 of bank g//64; bank A
    drains to DRAM while bank B still accumulates.
  - The scalar recurrence decays by 1/128 per step, so it is re-run exactly
    on the gathered per-row sums on host as a short causal conv in float64.
"""

import numpy as np

N, DIM, S = 500000, 128, 10
INV_SQRT_2PI = 0.3989422804014327
NCORES = 8
CHUNK = 512
NCHUNK = 123                     # chunks per core (123*512 = 62976 rows)
R = NCHUNK * CHUNK

# (width, act_cols, gpsimd_cols) per tile; dve_cols = width - act - gpsimd.
# All multiples of 512. Small head tiles so the engines start before the big
# DMAs land; the ACT:DVE:GPSIMD ratio balances the three producers
# (~0.88 / ~1.07 / ~4.2 ns per column per distribution respectively; GPSIMD
# computes the pre-clamp fp32 value and the DVE finishes it at 2x).
# Small final tile keeps the trailing matmul burst off the critical path.
TILE_SPECS = [
    (1024, 512, 0),
    (2048, 1024, 0),
    (3072, 1536, 0),
    (4096, 2048, 0),
    (6144, 3584, 0),
    (8192, 4608, 0),
    (11264, 6144, 0),
    (11264, 6144, 0),
    (11264, 6144, 0),
    (3584, 2048, 0),
    (1024, 512, 0),
]
assert sum(w for w, _, _ in TILE_SPECS) == R
assert all(w % CHUNK == 0 and wa % CHUNK == 0 and wg % CHUNK == 0
           and wa + wg <= w for w, wa, wg in TILE_SPECS)
MAX_W = max(w for w, _, _ in TILE_SPECS)
MAX_WA = max(wa for _, wa, _ in TILE_SPECS)
MAX_WV = max(w - wa - wg for w, wa, wg in TILE_SPECS)
MAX_WG = max(wg for _, _, wg in TILE_SPECS)

# fp16 exp2 bit-trick constants: bits = relu(C2F - (K*z)^2), K = sqrt(1024*log2 e)
# so bits ~ 1024*(15 + log2((2/sqrt(pi)) * exp(-z^2))), int16(bits).view(fp16)
# ~ (2/sqrt(pi))*exp(-z^2). corr centers the mantissa-linear ripple.
K2 = 1024 * np.log2(np.e)
KSQ = float(np.sqrt(K2))
C2F = float(1024 * (15 + np.log2(2.0 / np.sqrt(np.pi))) - 15.0)

_COMPILED = None
LAST_RESULTS = None


def _register_custom_op():
    from concourse.dve_spec import Spec, Src0, C0, C1, C2, relu, sq
    from concourse import dve_ops
    from concourse.dve_ops import DveOp

    for op in dve_ops.OPS:
        if op.name == "GAUSS_BITS_ANT":
            return op

    def _ref(in0, in1, s0, s1, imm2):
        y = in0.astype(np.float32) * s0 + s1
        return np.maximum(imm2 - y * y, 0).astype(np.float32)

    op = DveOp(
        "GAUSS_BITS_ANT",
        Spec(body=relu(C2 - sq(Src0 * C0 + C1)), reference=_ref),
        subdim=False,
        uops_sha={"v3": "c18ec8d64d08f78d", "v4": "8a0219efcf9878d5"},
    )
    dve_ops.OPS.append(op)
    dve_ops._SUB_OPCODE_FOR_NAME[op.name] = (
        max(dve_ops._SUB_OPCODE_FOR_NAME.values()) + 1
    )
    dve_ops.CUSTOM_DVE_SPECS[op.name] = op.spec
    return op


def _build():
    import concourse.tile as tile
    from concourse import bacc, mybir

    gauss_op = _register_custom_op()

    nc = bacc.Bacc("TRN2", target_bir_lowering=False, debug=False,
                   num_devices=NCORES)

    xT = nc.dram_tensor("xT", [DIM, R], mybir.dt.float16,
                        kind="ExternalInput").ap()
    # consts cols: 0 scale_n, 1 bias_n, 2 scale_a, 3 bias_a,
    #              4 A_n, 5 B_n, 6 A_a, 7 B_a
    consts = nc.dram_tensor("consts", [DIM, 8], mybir.dt.float32,
                            kind="ExternalInput").ap()
    # wmat: col 64 = w_n, col 192 = w_a, zeros elsewhere; stationary windows
    # [64-r, 128-r) / [192-r, 256-r) place the weight at relative col r.
    wmat = nc.dram_tensor("wmat", [DIM, 256], mybir.dt.float16,
                          kind="ExternalInput").ap()
    sn_out = nc.dram_tensor("sn_out", [128, CHUNK], mybir.dt.float16,
                            kind="ExternalOutput").ap()
    sa_out = nc.dram_tensor("sa_out", [128, CHUNK], mybir.dt.float16,
                            kind="ExternalOutput").ap()

    DErf = mybir.ActivationFunctionType.Derivative_Erf

    # tile index after which PSUM bank A (chunks 0-63) is complete
    FLUSH_AFTER_TILE = None
    cum = 0
    for ti, (w, _, _) in enumerate(TILE_SPECS):
        cum += w // CHUNK
        if cum >= 64 and FLUSH_AFTER_TILE is None:
            FLUSH_AFTER_TILE = ti

    with tile.TileContext(nc) as tc:
        with tc.tile_pool(name="cpool", bufs=1) as cpool, \
             tc.tile_pool(name="xpool", bufs=4) as xpool, \
             tc.tile_pool(name="apool", bufs=2) as apool, \
             tc.tile_pool(name="vpool", bufs=2) as vpool, \
             tc.tile_pool(name="gpool", bufs=2) as gpool, \
             tc.tile_pool(name="egpool", bufs=2) as egpool, \
             tc.tile_pool(name="pspool", bufs=1, space="PSUM") as pspool:

            # x tiles first on the Sync HWDGE queue: the first ACTIVATE is
            # gated on x0's DMA receipt, so nothing may precede it there.
            x_pre = {}
            for ti in (0, 1, 2, 3):
                off = sum(w for w, _, _ in TILE_SPECS[:ti])
                w = TILE_SPECS[ti][0]
                x_t = xpool.tile([DIM, w], mybir.dt.float16, tag="x",
                                 padded_shape=[DIM, MAX_W], name=f"x_pre{ti}")
                nc.sync.dma_start(x_t[:], xT[:, off:off + w])
                x_pre[ti] = x_t
            # consts + weights ride the GpSimd SWDGE queue
            consts_t = cpool.tile([DIM, 8], mybir.dt.float32)
            nc.gpsimd.dma_start(consts_t[:], consts[:, :])
            w_t = cpool.tile([DIM, 256], mybir.dt.float16)
            nc.gpsimd.dma_start(w_t[:], wmat[:, :])
            # table-load warmup gated only on a local memset, not any DMA
            warm_in = cpool.tile([DIM, 1], mybir.dt.float32)
            nc.vector.memset(warm_in[:], 0.0)
            warm_t = cpool.tile([DIM, 1], mybir.dt.float32)
            nc.scalar.activation(warm_t[:], warm_in[:], DErf,
                                 bias=0.0, scale=1.0)

            sn_psA = pspool.tile([64, CHUNK], mybir.dt.float32)
            sn_psB = pspool.tile([64, CHUNK], mybir.dt.float32)
            sa_psA = pspool.tile([64, CHUNK], mybir.dt.float32)
            sa_psB = pspool.tile([64, CHUNK], mybir.dt.float32)

            sn_sbA = cpool.tile([64, CHUNK], mybir.dt.float16)
            sa_sbA = cpool.tile([64, CHUNK], mybir.dt.float16)

            AL = mybir.AluOpType
            NK2 = float(-1024 * np.log2(np.e))
            g = 0
            off = 0
            for ti, (w, wa, wg) in enumerate(TILE_SPECS):
                wv = w - wa - wg
                if ti in x_pre:
                    x_t = x_pre[ti]
                else:
                    x_t = xpool.tile([DIM, w], mybir.dt.float16, tag="x",
                                     padded_shape=[DIM, MAX_W])
                    nc.sync.dma_start(x_t[:], xT[:, off:off + w])
                # ScalarE: exact Gaussians on columns [0, wa)
                ea_n = apool.tile([DIM, wa], mybir.dt.float16, tag="ean",
                                  padded_shape=[DIM, MAX_WA])
                nc.scalar.activation(ea_n[:], x_t[:, 0:wa], DErf,
                                     bias=consts_t[:, 1:2],
                                     scale=consts_t[:, 0:1])
                ea_a = apool.tile([DIM, wa], mybir.dt.float16, tag="eaa",
                                  padded_shape=[DIM, MAX_WA])
                nc.scalar.activation(ea_a[:], x_t[:, 0:wa], DErf,
                                     bias=consts_t[:, 3:4],
                                     scale=consts_t[:, 2:3])
                # VectorE: exp2 bit-trick Gaussians on columns [wa, wa+wv)
                ev_n = ev_a = None
                if wv:
                    ev_n = vpool.tile([DIM, wv], mybir.dt.float16, tag="evn",
                                      padded_shape=[DIM, MAX_WV])
                    nc.vector._custom_dve(
                        gauss_op, out=ev_n[:].bitcast(mybir.dt.int16),
                        in0=x_t[:, wa:wa + wv],
                        s0=consts_t[:, 4:5], s1=consts_t[:, 5:6], imm2=C2F)
                    ev_a = vpool.tile([DIM, wv], mybir.dt.float16, tag="eva",
                                      padded_shape=[DIM, MAX_WV])
                    nc.vector._custom_dve(
                        gauss_op, out=ev_a[:].bitcast(mybir.dt.int16),
                        in0=x_t[:, wa:wa + wv],
                        s0=consts_t[:, 6:7], s1=consts_t[:, 7:8], imm2=C2F)
                # GPSIMD: pre-clamp bits value on columns [wa+wv, w); the
                # VectorE finishes with max(v,0) -> int16 at 2x mode.
                eg_n = eg_a = None
                if wg:
                    gx = x_t[:, wa + wv:w]
                    eg = []
                    for sc, sb in ((0, 1), (2, 3)):
                        z_g = gpool.tile([DIM, wg], mybir.dt.float16,
                                         tag=f"zg{sc}", padded_shape=[DIM, MAX_WG])
                        nc.gpsimd.tensor_scalar(z_g[:], gx,
                                                consts_t[:, sc:sc + 1],
                                                consts_t[:, sb:sb + 1],
                                                AL.mult, AL.add)
                        q_g = gpool.tile([DIM, wg], mybir.dt.float32,
                                         tag=f"qg{sc}", padded_shape=[DIM, MAX_WG])
                        nc.gpsimd.tensor_tensor(q_g[:], z_g[:], z_g[:], AL.mult)
                        nc.gpsimd.tensor_scalar(q_g[:], q_g[:], NK2, C2F,
                                                AL.mult, AL.add)
                        e_g = egpool.tile([DIM, wg], mybir.dt.float16,
                                          tag=f"eg{sc}", padded_shape=[DIM, MAX_WG])
                        nc.vector.tensor_scalar_max(
                            e_g[:].bitcast(mybir.dt.int16), q_g[:], 0.0)
                        eg.append(e_g)
                    eg_n, eg_a = eg

                def rhs_for(lo, e_act, e_dve, e_gp):
                    if lo < wa:
                        return e_act[:, lo:lo + CHUNK]
                    if lo < wa + wv:
                        return e_dve[:, lo - wa:lo - wa + CHUNK]
                    return e_gp[:, lo - wa - wv:lo - wa - wv + CHUNK]

                # all sn chunks first: they only need the _n tiles, which
                # finish one whole producer-op earlier than the _a tiles
                for c in range(w // CHUNK):
                    gg = g + c
                    rr = gg % 64
                    rn = rhs_for(c * CHUNK, ea_n, ev_n, eg_n)
                    sn_ps = sn_psA if gg < 64 else sn_psB
                    nc.tensor.matmul(sn_ps[:], w_t[:, 64 - rr:128 - rr], rn,
                                     start=rr == 0,
                                     stop=gg == 63 or gg == NCHUNK - 1,
                                     skip_group_check=True)
                for c in range(w // CHUNK):
                    gg = g + c
                    rr = gg % 64
                    ra = rhs_for(c * CHUNK, ea_a, ev_a, eg_a)
                    sa_ps = sa_psA if gg < 64 else sa_psB
                    nc.tensor.matmul(sa_ps[:], w_t[:, 192 - rr:256 - rr], ra,
                                     start=rr == 0,
                                     stop=gg == 63 or gg == NCHUNK - 1,
                                     skip_group_check=True)
                g += w // CHUNK
                if ti == FLUSH_AFTER_TILE:
                    nc.scalar.copy(sn_sbA[:], sn_psA[:])
                    nc.vector.tensor_copy(sa_sbA[:], sa_psA[:])
                    nc.sync.dma_start(sn_out[0:64, :], sn_sbA[:])
                    nc.sync.dma_start(sa_out[0:64, :], sa_sbA[:])
                off += w

            sn_sbB = cpool.tile([64, CHUNK], mybir.dt.float16)
            nc.vector.tensor_copy(sn_sbB[:], sn_psB[:])
            sa_sbB = cpool.tile([64, CHUNK], mybir.dt.float16)
            nc.scalar.copy(sa_sbB[:], sa_psB[:])
            nc.sync.dma_start(sn_out[64:128, :], sn_sbB[:])
            nc.sync.dma_start(sa_out[64:128, :], sa_sbB[:])

    nc.compile()
    return nc


def _get_compiled():
    global _COMPILED
    if _COMPILED is None:
        _COMPILED = _build()
    return _COMPILED


def kernel(encoded, normal_dist, anomaly_dist):
    global LAST_RESULTS
    from concourse.bass_utils import run_bass_kernel_spmd

    x = np.asarray(encoded, dtype=np.float32)
    nd = np.asarray(normal_dist, dtype=np.float64)
    ad = np.asarray(anomaly_dist, dtype=np.float64)

    mu_n = nd.mean(axis=1)
    sd_n = nd.std(axis=1, ddof=1)
    mu_a = ad.mean(axis=1)
    sd_a = ad.std(axis=1, ddof=1)
    isd_n, isd_a = 1.0 / sd_n, 1.0 / sd_a

    inv_sqrt2 = 1.0 / np.sqrt(2.0)
    a_n = isd_n * inv_sqrt2
    b_n = -mu_n * isd_n * inv_sqrt2
    a_a = isd_a * inv_sqrt2
    b_a = -mu_a * isd_a * inv_sqrt2
    consts = np.stack([
        a_n, b_n, a_a, b_a,
        a_n * KSQ, b_n * KSQ, a_a * KSQ, b_a * KSQ,
    ], axis=1).astype(np.float32)      # [128, 8]

    half_sqrt_pi = 0.5 * np.sqrt(np.pi)
    c_n = (INV_SQRT_2PI * isd_n * half_sqrt_pi).astype(np.float16)
    c_a = (INV_SQRT_2PI * isd_a * half_sqrt_pi).astype(np.float16)
    wmat = np.zeros((DIM, 256), dtype=np.float16)
    wmat[:, 64] = c_n
    wmat[:, 192] = c_a

    x16 = x.astype(np.float16)
    in_maps = []
    for i in range(NCORES):
        lo = i * R
        hi = min(lo + R, N)
        shard_T = np.zeros((DIM, R), dtype=np.float16)
        shard_T[:, :hi - lo] = x16[lo:hi].T
        in_maps.append({"xT": shard_T, "consts": consts, "wmat": wmat})

    nc = _get_compiled()
    try:
        res = run_bass_kernel_spmd(nc, in_maps, core_ids=list(range(NCORES)))
    except Exception:
        # one retry: the NRT occasionally reports a transient
        # NRT_EXEC_UNIT_UNRECOVERABLE on an otherwise-healthy device
        res = run_bass_kernel_spmd(nc, in_maps, core_ids=list(range(NCORES)))
    LAST_RESULTS = res

    s_n = np.empty(N, dtype=np.float64)
    s_a = np.empty(N, dtype=np.float64)
    for i in range(NCORES):
        lo = i * R
        hi = min(lo + R, N)
        s_n[lo:hi] = res.results[i]["sn_out"].reshape(-1)[:hi - lo]
        s_a[lo:hi] = res.results[i]["sa_out"].reshape(-1)[:hi - lo]

    # exact recurrence p_k = (p_{k-1} + s_k)/dim as truncated causal
    # convolution: p_k = sum_j (1/dim)^(j+1) s_{k-j}; (1/128)^14 ~ 3e-30.
    a = 1.0 / DIM
    pn = np.zeros(N, dtype=np.float64)
    pa = np.zeros(N, dtype=np.float64)
    wgt = a
    for j in range(14):
        if j == 0:
            pn += wgt * s_n
            pa += wgt * s_a
        else:
            pn[j:] += wgt * s_n[:-j]
            pa[j:] += wgt * s_a[:-j]
        wgt *= a
    total = pn + pa
    out = np.empty((N, 2), dtype=np.float32)
    out[:, 0] = (pn / total).astype(np.float32)
    out[:, 1] = (pa / total).astype(np.float32)
    return out


# revision 40
# speedup vs baseline: 1.0152x; 1.0152x over previous
"""Trainium2 Bass kernel for nn_ClassificationLayer (Gaussian pdf-sum classifier).

Math:
  mu/sd per dim from tiny [128,10] reference sets (host, exact).
  Per row i: s_n[i] = sum_d INV_SQRT_2PI/sd_d * exp(-0.5*((x[i,d]-mu_d)/sd_d)^2)
  (same for anomaly), then the batch recurrence p_k = (p_{k-1} + s_k)/128,
  output = [pn/(pn+pa), pa/(pn+pa)].

Device strategy (8 cores, data-parallel over N):
  - Host transposes each core's row-shard to [128 dims, R rows] fp16 so
    per-dim constants become per-partition scale/bias, and HBM traffic
    halves vs fp32 (DMA floor ~45us/core).
  - Row-columns are split between two engines working concurrently:
      * ScalarE ACTIVATE Derivative_Erf(scale*x+bias) = (2/sqrt(pi))exp(-z^2)
        -> exact Gaussian, fp16 out (~0.88 ns/col/dist).
      * VectorE custom DVE op GAUSS_BITS_ANT: relu(C2 - (A*x+B)^2) -> int16,
        which IS the fp16 bit pattern of ~(2/sqrt(pi))exp(-z^2) (Schraudolph
        exp2 trick, ~3% element ripple that averages out over 128 dims;
        end-to-end rel err ~1e-3). One DVE instr per dist (~1.07 ns/col/dist).
  - Reduction over dims (partitions) via fp16 TensorE matvec; the stationary
    operand is a 64-wide shifted window over a zero-padded fp16 weight buffer
    so chunk g's sums land in PSUM partition g%64 of bank g//64; bank A
    drains to DRAM while bank B still accumulates.
  - The scalar recurrence decays by 1/128 per step, so it is re-run exactly
    on the gathered per-row sums on host as a short causal conv in float64.
"""

import numpy as np

N, DIM, S = 500000, 128, 10
INV_SQRT_2PI = 0.3989422804014327
NCORES = 8
CHUNK = 512
NCHUNK = 123                     # chunks per core (123*512 = 62976 rows)
R = NCHUNK * CHUNK

# (width, act_cols, gpsimd_cols) per tile; dve_cols = width - act - gpsimd.
# All multiples of 512. Small head tiles so the engines start before the big
# DMAs land; the ACT:DVE:GPSIMD ratio balances the three producers
# (~0.88 / ~1.07 / ~4.2 ns per column per distribution respectively; GPSIMD
# computes the pre-clamp fp32 value and the DVE finishes it at 2x).
# Small final tile keeps the trailing matmul burst off the critical path.
TILE_SPECS = [
    (1024, 512, 0),
    (2048, 1024, 0),
    (3072, 1536, 0),
    (4096, 2048, 0),
    (6144, 3584, 0),
    (8192, 4608, 0),
    (11264, 6144, 0),
    (11264, 6144, 0),
    (9728, 5120, 0),
    (4608, 2560, 0),
    (1536, 1024, 0),
]
assert sum(w for w, _, _ in TILE_SPECS) == R
assert all(w % CHUNK == 0 and wa % CHUNK == 0 and wg % CHUNK == 0
           and wa + wg <= w for w, wa, wg in TILE_SPECS)
MAX_W = max(w for w, _, _ in TILE_SPECS)
MAX_WA = max(wa for _, wa, _ in TILE_SPECS)
MAX_WV = max(w - wa - wg for w, wa, wg in TILE_SPECS)
MAX_WG = max(wg for _, _, wg in TILE_SPECS)

# fp16 exp2 bit-trick constants: bits = relu(C2F - (K*z)^2), K = sqrt(1024*log2 e)
# so bits ~ 1024*(15 + log2((2/sqrt(pi)) * exp(-z^2))), int16(bits).view(fp16)
# ~ (2/sqrt(pi))*exp(-z^2). corr centers the mantissa-linear ripple.
K2 = 1024 * np.log2(np.e)
KSQ = float(np.sqrt(K2))
C2F = float(1024 * (15 + np.log2(2.0 / np.sqrt(np.pi))) - 15.0)

_COMPILED = None
LAST_RESULTS = None


def _register_custom_op():
    from concourse.dve_spec import Spec, Src0, C0, C1, C2, relu, sq
    from concourse import dve_ops
    from concourse.dve_ops import DveOp

    for op in dve_ops.OPS:
        if op.name == "GAUSS_BITS_ANT":
            return op

    def _ref(in0, in1, s0, s1, imm2):
        y = in0.astype(np.float32) * s0 + s1
        return np.maximum(imm2 - y * y, 0).astype(np.float32)

    op = DveOp(
        "GAUSS_BITS_ANT",
        Spec(body=relu(C2 - sq(Src0 * C0 + C1)), reference=_ref),
        subdim=False,
        uops_sha={"v3": "c18ec8d64d08f78d", "v4": "8a0219efcf9878d5"},
    )
    dve_ops.OPS.append(op)
    dve_ops._SUB_OPCODE_FOR_NAME[op.name] = (
        max(dve_ops._SUB_OPCODE_FOR_NAME.values()) + 1
    )
    dve_ops.CUSTOM_DVE_SPECS[op.name] = op.spec
    return op


def _build():
    import concourse.tile as tile
    from concourse import bacc, mybir

    gauss_op = _register_custom_op()

    nc = bacc.Bacc("TRN2", target_bir_lowering=False, debug=False,
                   num_devices=NCORES)

    xT = nc.dram_tensor("xT", [DIM, R], mybir.dt.float16,
                        kind="ExternalInput").ap()
    # consts cols: 0 scale_n, 1 bias_n, 2 scale_a, 3 bias_a,
    #              4 A_n, 5 B_n, 6 A_a, 7 B_a
    consts = nc.dram_tensor("consts", [DIM, 8], mybir.dt.float32,
                            kind="ExternalInput").ap()
    # wmat: col 64 = w_n, col 192 = w_a, zeros elsewhere; stationary windows
    # [64-r, 128-r) / [192-r, 256-r) place the weight at relative col r.
    wmat = nc.dram_tensor("wmat", [DIM, 256], mybir.dt.float16,
                          kind="ExternalInput").ap()
    sn_out = nc.dram_tensor("sn_out", [128, CHUNK], mybir.dt.float32,
                            kind="ExternalOutput").ap()
    sa_out = nc.dram_tensor("sa_out", [128, CHUNK], mybir.dt.float32,
                            kind="ExternalOutput").ap()

    DErf = mybir.ActivationFunctionType.Derivative_Erf

    # tile index after which PSUM bank A (chunks 0-63) is complete
    FLUSH_AFTER_TILE = None
    cum = 0
    for ti, (w, _, _) in enumerate(TILE_SPECS):
        cum += w // CHUNK
        if cum >= 64 and FLUSH_AFTER_TILE is None:
            FLUSH_AFTER_TILE = ti

    with tile.TileContext(nc) as tc:
        with tc.tile_pool(name="cpool", bufs=1) as cpool, \
             tc.tile_pool(name="xpool", bufs=4) as xpool, \
             tc.tile_pool(name="apool", bufs=2) as apool, \
             tc.tile_pool(name="vpool", bufs=2) as vpool, \
             tc.tile_pool(name="gpool", bufs=2) as gpool, \
             tc.tile_pool(name="egpool", bufs=2) as egpool, \
             tc.tile_pool(name="pspool", bufs=1, space="PSUM") as pspool:

            # x tiles first on the Sync HWDGE queue: the first ACTIVATE is
            # gated on x0's DMA receipt, so nothing may precede it there.
            x_pre = {}
            for ti in (0, 1, 2, 3):
                off = sum(w for w, _, _ in TILE_SPECS[:ti])
                w = TILE_SPECS[ti][0]
                x_t = xpool.tile([DIM, w], mybir.dt.float16, tag="x",
                                 padded_shape=[DIM, MAX_W], name=f"x_pre{ti}")
                nc.sync.dma_start(x_t[:], xT[:, off:off + w])
                x_pre[ti] = x_t
            # consts + weights ride the GpSimd SWDGE queue
            consts_t = cpool.tile([DIM, 8], mybir.dt.float32)
            nc.gpsimd.dma_start(consts_t[:], consts[:, :])
            w_t = cpool.tile([DIM, 256], mybir.dt.float16)
            nc.gpsimd.dma_start(w_t[:], wmat[:, :])
            # table-load warmup gated only on a local memset, not any DMA
            warm_in = cpool.tile([DIM, 1], mybir.dt.float32)
            nc.vector.memset(warm_in[:], 0.0)
            warm_t = cpool.tile([DIM, 1], mybir.dt.float32)
            nc.scalar.activation(warm_t[:], warm_in[:], DErf,
                                 bias=0.0, scale=1.0)

            sn_psA = pspool.tile([64, CHUNK], mybir.dt.float32)
            sn_psB = pspool.tile([64, CHUNK], mybir.dt.float32)
            sa_psA = pspool.tile([64, CHUNK], mybir.dt.float32)
            sa_psB = pspool.tile([64, CHUNK], mybir.dt.float32)

            sn_sbA = cpool.tile([64, CHUNK], mybir.dt.float32)
            sa_sbA = cpool.tile([64, CHUNK], mybir.dt.float32)

            AL = mybir.AluOpType
            NK2 = float(-1024 * np.log2(np.e))
            g = 0
            off = 0
            for ti, (w, wa, wg) in enumerate(TILE_SPECS):
                wv = w - wa - wg
                if ti in x_pre:
                    x_t = x_pre[ti]
                else:
                    x_t = xpool.tile([DIM, w], mybir.dt.float16, tag="x",
                                     padded_shape=[DIM, MAX_W])
                    nc.sync.dma_start(x_t[:], xT[:, off:off + w])
                # ScalarE: exact Gaussians on columns [0, wa)
                ea_n = apool.tile([DIM, wa], mybir.dt.float16, tag="ean",
                                  padded_shape=[DIM, MAX_WA])
                nc.scalar.activation(ea_n[:], x_t[:, 0:wa], DErf,
                                     bias=consts_t[:, 1:2],
                                     scale=consts_t[:, 0:1])
                ea_a = apool.tile([DIM, wa], mybir.dt.float16, tag="eaa",
                                  padded_shape=[DIM, MAX_WA])
                nc.scalar.activation(ea_a[:], x_t[:, 0:wa], DErf,
                                     bias=consts_t[:, 3:4],
                                     scale=consts_t[:, 2:3])
                # VectorE: exp2 bit-trick Gaussians on columns [wa, wa+wv)
                ev_n = ev_a = None
                if wv:
                    ev_n = vpool.tile([DIM, wv], mybir.dt.float16, tag="evn",
                                      padded_shape=[DIM, MAX_WV])
                    nc.vector._custom_dve(
                        gauss_op, out=ev_n[:].bitcast(mybir.dt.int16),
                        in0=x_t[:, wa:wa + wv],
                        s0=consts_t[:, 4:5], s1=consts_t[:, 5:6], imm2=C2F)
                    ev_a = vpool.tile([DIM, wv], mybir.dt.float16, tag="eva",
                                      padded_shape=[DIM, MAX_WV])
                    nc.vector._custom_dve(
                        gauss_op, out=ev_a[:].bitcast(mybir.dt.int16),
                        in0=x_t[:, wa:wa + wv],
                        s0=consts_t[:, 6:7], s1=consts_t[:, 7:8], imm2=C2F)
                # GPSIMD: pre-clamp bits value on columns [wa+wv, w); the
                # VectorE finishes with max(v,0) -> int16 at 2x mode.
                eg_n = eg_a = None
                if wg:
                    gx = x_t[:, wa + wv:w]
                    eg = []
                    for sc, sb in ((0, 1), (2, 3)):
                        z_g = gpool.tile([DIM, wg], mybir.dt.float16,
                                         tag=f"zg{sc}", padded_shape=[DIM, MAX_WG])
                        nc.gpsimd.tensor_scalar(z_g[:], gx,
                                                consts_t[:, sc:sc + 1],
                                                consts_t[:, sb:sb + 1],
                                                AL.mult, AL.add)
                        q_g = gpool.tile([DIM, wg], mybir.dt.float32,
                                         tag=f"qg{sc}", padded_shape=[DIM, MAX_WG])
                        nc.gpsimd.tensor_tensor(q_g[:], z_g[:], z_g[:], AL.mult)
                        nc.gpsimd.tensor_scalar(q_g[:], q_g[:], NK2, C2F,
                                                AL.mult, AL.add)
                        e_g = egpool.tile([DIM, wg], mybir.dt.float16,
                                          tag=f"eg{sc}", padded_shape=[DIM, MAX_WG])
                        nc.vector.tensor_scalar_max(
                            e_g[:].bitcast(mybir.dt.int16), q_g[:], 0.0)
                        eg.append(e_g)
                    eg_n, eg_a = eg

                def rhs_for(lo, e_act, e_dve, e_gp):
                    if lo < wa:
                        return e_act[:, lo:lo + CHUNK]
                    if lo < wa + wv:
                        return e_dve[:, lo - wa:lo - wa + CHUNK]
                    return e_gp[:, lo - wa - wv:lo - wa - wv + CHUNK]

                # all sn chunks first: they only need the _n tiles, which
                # finish one whole producer-op earlier than the _a tiles
                for c in range(w // CHUNK):
                    gg = g + c
                    rr = gg % 64
                    rn = rhs_for(c * CHUNK, ea_n, ev_n, eg_n)
                    sn_ps = sn_psA if gg < 64 else sn_psB
                    nc.tensor.matmul(sn_ps[:], w_t[:, 64 - rr:128 - rr], rn,
                                     start=rr == 0,
                                     stop=gg == 63 or gg == NCHUNK - 1,
                                     skip_group_check=True)
                for c in range(w // CHUNK):
                    gg = g + c
                    rr = gg % 64
                    ra = rhs_for(c * CHUNK, ea_a, ev_a, eg_a)
                    sa_ps = sa_psA if gg < 64 else sa_psB
                    nc.tensor.matmul(sa_ps[:], w_t[:, 192 - rr:256 - rr], ra,
                                     start=rr == 0,
                                     stop=gg == 63 or gg == NCHUNK - 1,
                                     skip_group_check=True)
                g += w // CHUNK
                if ti == FLUSH_AFTER_TILE:
                    nc.scalar.copy(sn_sbA[:], sn_psA[:])
                    nc.vector.tensor_copy(sa_sbA[:], sa_psA[:])
                    nc.sync.dma_start(sn_out[0:64, :], sn_sbA[:])
                    nc.sync.dma_start(sa_out[0:64, :], sa_sbA[:])
                off += w

            sn_sbB = cpool.tile([64, CHUNK], mybir.dt.float32)
            nc.vector.tensor_copy(sn_sbB[:], sn_psB[:])
            sa_sbB = cpool.tile([64, CHUNK], mybir.dt.float32)
            nc.scalar.copy(sa_sbB[:], sa_psB[:])
            nc.sync.dma_start(sn_out[64:128, :], sn_sbB[:])
            nc.sync.dma_start(sa_out[64:128, :], sa_sbB[:])

    nc.compile()
    return nc


def _get_compiled():
    global _COMPILED
    if _COMPILED is None:
        _COMPILED = _build()
    return _COMPILED


def kernel(encoded, normal_dist, anomaly_dist):
    global LAST_RESULTS
    from concourse.bass_utils import run_bass_kernel_spmd

    x = np.asarray(encoded, dtype=np.float32)
    nd = np.asarray(normal_dist, dtype=np.float64)
    ad = np.asarray(anomaly_dist, dtype=np.float64)

    mu_n = nd.mean(axis=1)
    sd_n = nd.std(axis=1, ddof=1)
    mu_a = ad.mean(axis=1)
    sd_a = ad.std(axis=1, ddof=1)
    isd_n, isd_a = 1.0 / sd_n, 1.0 / sd_a

    inv_sqrt2 = 1.0 / np.sqrt(2.0)
    a_n = isd_n * inv_sqrt2
    b_n = -mu_n * isd_n * inv_sqrt2
    a_a = isd_a * inv_sqrt2
    b_a = -mu_a * isd_a * inv_sqrt2
    consts = np.stack([
        a_n, b_n, a_a, b_a,
        a_n * KSQ, b_n * KSQ, a_a * KSQ, b_a * KSQ,
    ], axis=1).astype(np.float32)      # [128, 8]

    half_sqrt_pi = 0.5 * np.sqrt(np.pi)
    c_n = (INV_SQRT_2PI * isd_n * half_sqrt_pi).astype(np.float16)
    c_a = (INV_SQRT_2PI * isd_a * half_sqrt_pi).astype(np.float16)
    wmat = np.zeros((DIM, 256), dtype=np.float16)
    wmat[:, 64] = c_n
    wmat[:, 192] = c_a

    x16 = x.astype(np.float16)
    in_maps = []
    for i in range(NCORES):
        lo = i * R
        hi = min(lo + R, N)
        shard_T = np.zeros((DIM, R), dtype=np.float16)
        shard_T[:, :hi - lo] = x16[lo:hi].T
        in_maps.append({"xT": shard_T, "consts": consts, "wmat": wmat})

    nc = _get_compiled()
    try:
        res = run_bass_kernel_spmd(nc, in_maps, core_ids=list(range(NCORES)))
    except Exception:
        # one retry: the NRT occasionally reports a transient
        # NRT_EXEC_UNIT_UNRECOVERABLE on an otherwise-healthy device
        res = run_bass_kernel_spmd(nc, in_maps, core_ids=list(range(NCORES)))
    LAST_RESULTS = res

    s_n = np.empty(N, dtype=np.float64)
    s_a = np.empty(N, dtype=np.float64)
    for i in range(NCORES):
        lo = i * R
        hi = min(lo + R, N)
        s_n[lo:hi] = res.results[i]["sn_out"].reshape(-1)[:hi - lo]
        s_a[lo:hi] = res.results[i]["sa_out"].reshape(-1)[:hi - lo]

    # exact recurrence p_k = (p_{k-1} + s_k)/dim as truncated causal
    # convolution: p_k = sum_j (1/dim)^(j+1) s_{k-j}; (1/128)^14 ~ 3e-30.
    a = 1.0 / DIM
    pn = np.zeros(N, dtype=np.float64)
    pa = np.zeros(N, dtype=np.float64)
    wgt = a
    for j in range(14):
        if j == 0:
            pn += wgt * s_n
            pa += wgt * s_a
        else:
            pn[j:] += wgt * s_n[:-j]
            pa[j:] += wgt * s_a[:-j]
        wgt *= a
    total = pn + pa
    out = np.empty((N, 2), dtype=np.float32)
    out[:, 0] = (pn / total).astype(np.float32)
    out[:, 1] = (pa / total).astype(np.float32)
    return out


# revision 41
# speedup vs baseline: 1.0206x; 1.0054x over previous
"""Trainium2 Bass kernel for nn_ClassificationLayer (Gaussian pdf-sum classifier).

Math:
  mu/sd per dim from tiny [128,10] reference sets (host, exact).
  Per row i: s_n[i] = sum_d INV_SQRT_2PI/sd_d * exp(-0.5*((x[i,d]-mu_d)/sd_d)^2)
  (same for anomaly), then the batch recurrence p_k = (p_{k-1} + s_k)/128,
  output = [pn/(pn+pa), pa/(pn+pa)].

Device strategy (8 cores, data-parallel over N):
  - Host transposes each core's row-shard to [128 dims, R rows] fp16 so
    per-dim constants become per-partition scale/bias, and HBM traffic
    halves vs fp32 (DMA floor ~45us/core).
  - Row-columns are split between two engines working concurrently:
      * ScalarE ACTIVATE Derivative_Erf(scale*x+bias) = (2/sqrt(pi))exp(-z^2)
        -> exact Gaussian, fp16 out (~0.88 ns/col/dist).
      * VectorE custom DVE op GAUSS_BITS_ANT: relu(C2 - (A*x+B)^2) -> int16,
        which IS the fp16 bit pattern of ~(2/sqrt(pi))exp(-z^2) (Schraudolph
        exp2 trick, ~3% element ripple that averages out over 128 dims;
        end-to-end rel err ~1e-3). One DVE instr per dist (~1.07 ns/col/dist).
  - Reduction over dims (partitions) via fp16 TensorE matvec; the stationary
    operand is a 64-wide shifted window over a zero-padded fp16 weight buffer
    so chunk g's sums land in PSUM partition g%64 of bank g//64; bank A
    drains to DRAM while bank B still accumulates.
  - The scalar recurrence decays by 1/128 per step, so it is re-run exactly
    on the gathered per-row sums on host as a short causal conv in float64.
"""

import numpy as np

N, DIM, S = 500000, 128, 10
INV_SQRT_2PI = 0.3989422804014327
NCORES = 8
CHUNK = 512
NCHUNK = 123                     # chunks per core (123*512 = 62976 rows)
R = NCHUNK * CHUNK

# (width, act_cols, gpsimd_cols) per tile; dve_cols = width - act - gpsimd.
# All multiples of 512. Small head tiles so the engines start before the big
# DMAs land; the ACT:DVE:GPSIMD ratio balances the three producers
# (~0.88 / ~1.07 / ~4.2 ns per column per distribution respectively; GPSIMD
# computes the pre-clamp fp32 value and the DVE finishes it at 2x).
# Small final tile keeps the trailing matmul burst off the critical path.
TILE_SPECS = [
    (1024, 512, 0),
    (2048, 1024, 0),
    (3072, 1536, 0),
    (4096, 2048, 0),
    (6144, 3584, 0),
    (8192, 4608, 0),
    (11264, 6144, 0),
    (11264, 6144, 0),
    (9728, 5120, 0),
    (4608, 2560, 0),
    (1536, 1024, 0),
]
assert sum(w for w, _, _ in TILE_SPECS) == R
assert all(w % CHUNK == 0 and wa % CHUNK == 0 and wg % CHUNK == 0
           and wa + wg <= w for w, wa, wg in TILE_SPECS)
MAX_W = max(w for w, _, _ in TILE_SPECS)
MAX_WA = max(wa for _, wa, _ in TILE_SPECS)
MAX_WV = max(w - wa - wg for w, wa, wg in TILE_SPECS)
MAX_WG = max(wg for _, _, wg in TILE_SPECS)

# fp16 exp2 bit-trick constants: bits = relu(C2F - (K*z)^2), K = sqrt(1024*log2 e)
# so bits ~ 1024*(15 + log2((2/sqrt(pi)) * exp(-z^2))), int16(bits).view(fp16)
# ~ (2/sqrt(pi))*exp(-z^2). corr centers the mantissa-linear ripple.
K2 = 1024 * np.log2(np.e)
KSQ = float(np.sqrt(K2))
C2F = float(1024 * (15 + np.log2(2.0 / np.sqrt(np.pi))) - 15.0)

_COMPILED = None
LAST_RESULTS = None


def _register_custom_op():
    from concourse.dve_spec import Spec, Src0, C0, C1, C2, relu, sq
    from concourse import dve_ops
    from concourse.dve_ops import DveOp

    for op in dve_ops.OPS:
        if op.name == "GAUSS_BITS_ANT":
            return op

    def _ref(in0, in1, s0, s1, imm2):
        y = in0.astype(np.float32) * s0 + s1
        return np.maximum(imm2 - y * y, 0).astype(np.float32)

    op = DveOp(
        "GAUSS_BITS_ANT",
        Spec(body=relu(C2 - sq(Src0 * C0 + C1)), reference=_ref),
        subdim=False,
        uops_sha={"v3": "c18ec8d64d08f78d", "v4": "8a0219efcf9878d5"},
    )
    dve_ops.OPS.append(op)
    dve_ops._SUB_OPCODE_FOR_NAME[op.name] = (
        max(dve_ops._SUB_OPCODE_FOR_NAME.values()) + 1
    )
    dve_ops.CUSTOM_DVE_SPECS[op.name] = op.spec
    return op


def _build():
    import concourse.tile as tile
    from concourse import bacc, mybir

    gauss_op = _register_custom_op()

    nc = bacc.Bacc("TRN2", target_bir_lowering=False, debug=False,
                   num_devices=NCORES)

    xT = nc.dram_tensor("xT", [DIM, R], mybir.dt.float16,
                        kind="ExternalInput").ap()
    # consts cols: 0 scale_n, 1 bias_n, 2 scale_a, 3 bias_a,
    #              4 A_n, 5 B_n, 6 A_a, 7 B_a
    consts = nc.dram_tensor("consts", [DIM, 8], mybir.dt.float32,
                            kind="ExternalInput").ap()
    # wmat: col 64 = w_n, col 192 = w_a, zeros elsewhere; stationary windows
    # [64-r, 128-r) / [192-r, 256-r) place the weight at relative col r.
    wmat = nc.dram_tensor("wmat", [DIM, 256], mybir.dt.float16,
                          kind="ExternalInput").ap()
    sn_out = nc.dram_tensor("sn_out", [128, CHUNK], mybir.dt.float32,
                            kind="ExternalOutput").ap()
    sa_out = nc.dram_tensor("sa_out", [128, CHUNK], mybir.dt.float32,
                            kind="ExternalOutput").ap()

    DErf = mybir.ActivationFunctionType.Derivative_Erf

    # tile index after which PSUM bank A (chunks 0-63) is complete
    FLUSH_AFTER_TILE = None
    cum = 0
    for ti, (w, _, _) in enumerate(TILE_SPECS):
        cum += w // CHUNK
        if cum >= 64 and FLUSH_AFTER_TILE is None:
            FLUSH_AFTER_TILE = ti

    with tile.TileContext(nc) as tc:
        with tc.tile_pool(name="cpool", bufs=1) as cpool, \
             tc.tile_pool(name="xpool", bufs=4) as xpool, \
             tc.tile_pool(name="apool", bufs=2) as apool, \
             tc.tile_pool(name="vpool", bufs=2) as vpool, \
             tc.tile_pool(name="gpool", bufs=2) as gpool, \
             tc.tile_pool(name="egpool", bufs=2) as egpool, \
             tc.tile_pool(name="pspool", bufs=1, space="PSUM") as pspool:

            # x0 first on the Sync HWDGE queue (the first ACTIVATE is gated on
            # its receipt), then the tiny consts (SWDGE completion semaphores
            # fire microseconds late, which stalled the first VectorE op),
            # then the remaining prefetches.
            x_pre = {}
            consts_t = cpool.tile([DIM, 8], mybir.dt.float32)
            for ti in (0, 1, 2, 3):
                off = sum(w for w, _, _ in TILE_SPECS[:ti])
                w = TILE_SPECS[ti][0]
                x_t = xpool.tile([DIM, w], mybir.dt.float16, tag="x",
                                 padded_shape=[DIM, MAX_W], name=f"x_pre{ti}")
                nc.sync.dma_start(x_t[:], xT[:, off:off + w])
                x_pre[ti] = x_t
                if ti == 0:
                    nc.sync.dma_start(consts_t[:], consts[:, :])
            # weights ride the GpSimd SWDGE queue (only matmuls need them)
            w_t = cpool.tile([DIM, 256], mybir.dt.float16)
            nc.gpsimd.dma_start(w_t[:], wmat[:, :])
            # table-load warmup gated only on a local memset, not any DMA
            warm_in = cpool.tile([DIM, 1], mybir.dt.float32)
            nc.vector.memset(warm_in[:], 0.0)
            warm_t = cpool.tile([DIM, 1], mybir.dt.float32)
            nc.scalar.activation(warm_t[:], warm_in[:], DErf,
                                 bias=0.0, scale=1.0)

            sn_psA = pspool.tile([64, CHUNK], mybir.dt.float32)
            sn_psB = pspool.tile([64, CHUNK], mybir.dt.float32)
            sa_psA = pspool.tile([64, CHUNK], mybir.dt.float32)
            sa_psB = pspool.tile([64, CHUNK], mybir.dt.float32)

            sn_sbA = cpool.tile([64, CHUNK], mybir.dt.float32)
            sa_sbA = cpool.tile([64, CHUNK], mybir.dt.float32)

            AL = mybir.AluOpType
            NK2 = float(-1024 * np.log2(np.e))
            g = 0
            off = 0
            for ti, (w, wa, wg) in enumerate(TILE_SPECS):
                wv = w - wa - wg
                if ti in x_pre:
                    x_t = x_pre[ti]
                else:
                    x_t = xpool.tile([DIM, w], mybir.dt.float16, tag="x",
                                     padded_shape=[DIM, MAX_W])
                    nc.sync.dma_start(x_t[:], xT[:, off:off + w])
                # ScalarE: exact Gaussians on columns [0, wa)
                ea_n = apool.tile([DIM, wa], mybir.dt.float16, tag="ean",
                                  padded_shape=[DIM, MAX_WA])
                nc.scalar.activation(ea_n[:], x_t[:, 0:wa], DErf,
                                     bias=consts_t[:, 1:2],
                                     scale=consts_t[:, 0:1])
                ea_a = apool.tile([DIM, wa], mybir.dt.float16, tag="eaa",
                                  padded_shape=[DIM, MAX_WA])
                nc.scalar.activation(ea_a[:], x_t[:, 0:wa], DErf,
                                     bias=consts_t[:, 3:4],
                                     scale=consts_t[:, 2:3])
                # VectorE: exp2 bit-trick Gaussians on columns [wa, wa+wv)
                ev_n = ev_a = None
                if wv:
                    ev_n = vpool.tile([DIM, wv], mybir.dt.float16, tag="evn",
                                      padded_shape=[DIM, MAX_WV])
                    nc.vector._custom_dve(
                        gauss_op, out=ev_n[:].bitcast(mybir.dt.int16),
                        in0=x_t[:, wa:wa + wv],
                        s0=consts_t[:, 4:5], s1=consts_t[:, 5:6], imm2=C2F)
                    ev_a = vpool.tile([DIM, wv], mybir.dt.float16, tag="eva",
                                      padded_shape=[DIM, MAX_WV])
                    nc.vector._custom_dve(
                        gauss_op, out=ev_a[:].bitcast(mybir.dt.int16),
                        in0=x_t[:, wa:wa + wv],
                        s0=consts_t[:, 6:7], s1=consts_t[:, 7:8], imm2=C2F)
                # GPSIMD: pre-clamp bits value on columns [wa+wv, w); the
                # VectorE finishes with max(v,0) -> int16 at 2x mode.
                eg_n = eg_a = None
                if wg:
                    gx = x_t[:, wa + wv:w]
                    eg = []
                    for sc, sb in ((0, 1), (2, 3)):
                        z_g = gpool.tile([DIM, wg], mybir.dt.float16,
                                         tag=f"zg{sc}", padded_shape=[DIM, MAX_WG])
                        nc.gpsimd.tensor_scalar(z_g[:], gx,
                                                consts_t[:, sc:sc + 1],
                                                consts_t[:, sb:sb + 1],
                                                AL.mult, AL.add)
                        q_g = gpool.tile([DIM, wg], mybir.dt.float32,
                                         tag=f"qg{sc}", padded_shape=[DIM, MAX_WG])
                        nc.gpsimd.tensor_tensor(q_g[:], z_g[:], z_g[:], AL.mult)
                        nc.gpsimd.tensor_scalar(q_g[:], q_g[:], NK2, C2F,
                                                AL.mult, AL.add)
                        e_g = egpool.tile([DIM, wg], mybir.dt.float16,
                                          tag=f"eg{sc}", padded_shape=[DIM, MAX_WG])
                        nc.vector.tensor_scalar_max(
                            e_g[:].bitcast(mybir.dt.int16), q_g[:], 0.0)
                        eg.append(e_g)
                    eg_n, eg_a = eg

                def rhs_for(lo, e_act, e_dve, e_gp):
                    if lo < wa:
                        return e_act[:, lo:lo + CHUNK]
                    if lo < wa + wv:
                        return e_dve[:, lo - wa:lo - wa + CHUNK]
                    return e_gp[:, lo - wa - wv:lo - wa - wv + CHUNK]

                # all sn chunks first: they only need the _n tiles, which
                # finish one whole producer-op earlier than the _a tiles
                for c in range(w // CHUNK):
                    gg = g + c
                    rr = gg % 64
                    rn = rhs_for(c * CHUNK, ea_n, ev_n, eg_n)
                    sn_ps = sn_psA if gg < 64 else sn_psB
                    nc.tensor.matmul(sn_ps[:], w_t[:, 64 - rr:128 - rr], rn,
                                     start=rr == 0,
                                     stop=gg == 63 or gg == NCHUNK - 1,
                                     skip_group_check=True)
                for c in range(w // CHUNK):
                    gg = g + c
                    rr = gg % 64
                    ra = rhs_for(c * CHUNK, ea_a, ev_a, eg_a)
                    sa_ps = sa_psA if gg < 64 else sa_psB
                    nc.tensor.matmul(sa_ps[:], w_t[:, 192 - rr:256 - rr], ra,
                                     start=rr == 0,
                                     stop=gg == 63 or gg == NCHUNK - 1,
                                     skip_group_check=True)
                g += w // CHUNK
                if ti == FLUSH_AFTER_TILE:
                    nc.scalar.copy(sn_sbA[:], sn_psA[:])
                    nc.vector.tensor_copy(sa_sbA[:], sa_psA[:])
                    nc.sync.dma_start(sn_out[0:64, :], sn_sbA[:])
                    nc.sync.dma_start(sa_out[0:64, :], sa_sbA[:])
                off += w

            sn_sbB = cpool.tile([64, CHUNK], mybir.dt.float32)
            nc.vector.tensor_copy(sn_sbB[:], sn_psB[:])
            sa_sbB = cpool.tile([64, CHUNK], mybir.dt.float32)
            nc.scalar.copy(sa_sbB[:], sa_psB[:])
            nc.sync.dma_start(sn_out[64:128, :], sn_sbB[:])
            nc.sync.dma_start(sa_out[64:128, :], sa_sbB[:])

    nc.compile()
    return nc


def _get_compiled():
    global _COMPILED
    if _COMPILED is None:
        _COMPILED = _build()
    return _COMPILED


def kernel(encoded, normal_dist, anomaly_dist):
    global LAST_RESULTS
    from concourse.bass_utils import run_bass_kernel_spmd

    x = np.asarray(encoded, dtype=np.float32)
    nd = np.asarray(normal_dist, dtype=np.float64)
    ad = np.asarray(anomaly_dist, dtype=np.float64)

    mu_n = nd.mean(axis=1)
    sd_n = nd.std(axis=1, ddof=1)
    mu_a = ad.mean(axis=1)
    sd_a = ad.std(axis=1, ddof=1)
    isd_n, isd_a = 1.0 / sd_n, 1.0 / sd_a

    inv_sqrt2 = 1.0 / np.sqrt(2.0)
    a_n = isd_n * inv_sqrt2
    b_n = -mu_n * isd_n * inv_sqrt2
    a_a = isd_a * inv_sqrt2
    b_a = -mu_a * isd_a * inv_sqrt2
    consts = np.stack([
        a_n, b_n, a_a, b_a,
        a_n * KSQ, b_n * KSQ, a_a * KSQ, b_a * KSQ,
    ], axis=1).astype(np.float32)      # [128, 8]

    half_sqrt_pi = 0.5 * np.sqrt(np.pi)
    c_n = (INV_SQRT_2PI * isd_n * half_sqrt_pi).astype(np.float16)
    c_a = (INV_SQRT_2PI * isd_a * half_sqrt_pi).astype(np.float16)
    wmat = np.zeros((DIM, 256), dtype=np.float16)
    wmat[:, 64] = c_n
    wmat[:, 192] = c_a

    x16 = x.astype(np.float16)
    in_maps = []
    for i in range(NCORES):
        lo = i * R
        hi = min(lo + R, N)
        shard_T = np.zeros((DIM, R), dtype=np.float16)
        shard_T[:, :hi - lo] = x16[lo:hi].T
        in_maps.append({"xT": shard_T, "consts": consts, "wmat": wmat})

    nc = _get_compiled()
    try:
        res = run_bass_kernel_spmd(nc, in_maps, core_ids=list(range(NCORES)))
    except Exception:
        # one retry: the NRT occasionally reports a transient
        # NRT_EXEC_UNIT_UNRECOVERABLE on an otherwise-healthy device
        res = run_bass_kernel_spmd(nc, in_maps, core_ids=list(range(NCORES)))
    LAST_RESULTS = res

    s_n = np.empty(N, dtype=np.float64)
    s_a = np.empty(N, dtype=np.float64)
    for i in range(NCORES):
        lo = i * R
        hi = min(lo + R, N)
        s_n[lo:hi] = res.results[i]["sn_out"].reshape(-1)[:hi - lo]
        s_a[lo:hi] = res.results[i]["sa_out"].reshape(-1)[:hi - lo]

    # exact recurrence p_k = (p_{k-1} + s_k)/dim as truncated causal
    # convolution: p_k = sum_j (1/dim)^(j+1) s_{k-j}; (1/128)^14 ~ 3e-30.
    a = 1.0 / DIM
    pn = np.zeros(N, dtype=np.float64)
    pa = np.zeros(N, dtype=np.float64)
    wgt = a
    for j in range(14):
        if j == 0:
            pn += wgt * s_n
            pa += wgt * s_a
        else:
            pn[j:] += wgt * s_n[:-j]
            pa[j:] += wgt * s_a[:-j]
        wgt *= a
    total = pn + pa
    out = np.empty((N, 2), dtype=np.float32)
    out[:, 0] = (pn / total).astype(np.float32)
    out[:, 1] = (pa / total).astype(np.float32)
    return out
